# revision 8
# baseline (speedup 1.0000x reference)
"""Multi-head attention block (QKV proj + causal-multiplicative-mask softmax
attention + out proj + residual + LayerNorm) on 8 Trainium2 NeuronCores.

Sharding: tensor-parallel over heads. Each core computes 2 of the 16 heads
end-to-end (QKV projections for its 128 feature columns, full attention for
its heads over all batches, and its slice of the output projection), then a
per-batch ReduceScatter sums the partial projection outputs and hands each
core a contiguous row shard on which it applies residual + LayerNorm.

The multiplicative causal mask (zeros above the diagonal, NOT -inf) means
softmax weights for j > i are exp(0) = 1.  Those contributions are the
suffix-sums of V, which we fold into the attention matmuls analytically:
  - V is stored with a ones-column per head (vh65), so the unnormalized
    context matmul also accumulates the softmax denominator.
  - a strict-upper-triangular ones matmul adds the within-diagonal-block
    j > i contributions (both numerator and denominator count).
  - per-block column-sums of vh65 are combined with a static selection
    matrix to add the contributions of all fully-masked blocks above the
    diagonal block.
This halves the score/exp/AV work vs. computing the full S x S attention.
"""

import numpy as np
import ml_dtypes

import concourse.bacc as bacc
import concourse.bass as bass
import concourse.mybir as mybir
import concourse.tile as tile
from concourse.bass_utils import run_bass_kernel_spmd

BF16 = ml_dtypes.bfloat16
F32 = mybir.dt.float32
BF = mybir.dt.bfloat16

B, S, D = 4, 2048, 1024
H, HD = 16, 64
SCALE = float(HD) ** 0.5
LN_EPS = 1e-5

NCORES = 8
HPC = H // NCORES          # heads per core = 2
FPC = HPC * HD             # feature cols per core = 128
NB = S // 128              # 16 in-batch row blocks of 128
RPC = B * S // NCORES      # 1024 output rows per core
RPB = S // NCORES          # 256 output rows per core per batch

_CACHE = {}


def _build_nc():
    nc = bacc.Bacc("TRN2", target_bir_lowering=False, debug=False,
                   num_devices=NCORES)

    # ---- I/O ----
    xqt = nc.dram_tensor("xqt", [D, B * S], BF, kind="ExternalInput")
    xkt = nc.dram_tensor("xkt", [D, B * S], BF, kind="ExternalInput")
    xvt = nc.dram_tensor("xvt", [D, B * S], BF, kind="ExternalInput")
    wq = nc.dram_tensor("wq", [D, FPC], BF, kind="ExternalInput")
    wk = nc.dram_tensor("wk", [D, FPC], BF, kind="ExternalInput")
    wv = nc.dram_tensor("wv", [D, FPC], BF, kind="ExternalInput")
    wp = nc.dram_tensor("wp", [FPC, D], BF, kind="ExternalInput")
    bqs = nc.dram_tensor("bqs", [FPC, 1], F32, kind="ExternalInput")
    bks = nc.dram_tensor("bks", [FPC, 1], F32, kind="ExternalInput")
    bvs = nc.dram_tensor("bvs", [1, FPC], F32, kind="ExternalInput")
    bp = nc.dram_tensor("bp", [1, D], F32, kind="ExternalInput")
    gam = nc.dram_tensor("gam", [1, D], F32, kind="ExternalInput")
    bet = nc.dram_tensor("bet", [1, D], F32, kind="ExternalInput")
    res = nc.dram_tensor("res", [RPC, D], F32, kind="ExternalInput")
    maskt = nc.dram_tensor("maskt", [128, 128], BF, kind="ExternalInput")
    triut = nc.dram_tensor("triut", [128, 128], BF, kind="ExternalInput")
    selb = nc.dram_tensor("selb", [NB, NB * 128], BF, kind="ExternalInput")
    ind16 = nc.dram_tensor("ind16", [128, NB * NB], BF, kind="ExternalInput")
    ident = nc.dram_tensor("ident", [128, 128], F32, kind="ExternalInput")
    y = nc.dram_tensor("y", [RPC, D], F32, kind="ExternalOutput")

    with tile.TileContext(nc) as tc:
        with tc.tile_pool(name="consts", bufs=1) as cpool, \
                tc.tile_pool(name="xbig", bufs=1) as xbig, \
                tc.tile_pool(name="kv", bufs=2) as kvp, \
                tc.tile_pool(name="exp", bufs=4) as epool, \
                tc.tile_pool(name="ctx", bufs=2) as ctxp, \
                tc.tile_pool(name="f32big", bufs=6) as fpool, \
                tc.tile_pool(name="small", bufs=6) as smallp, \
                tc.tile_pool(name="dram", bufs=2, space="DRAM") as dram, \
                tc.tile_pool(name="ps_scores", bufs=2, space="PSUM") as spool, \
                tc.tile_pool(name="ps_av", bufs=2, space="PSUM") as apool, \
                tc.tile_pool(name="ps_mm", bufs=2, space="PSUM") as ppool:
            # ---- constants ----
            def cload(src, shape, dtype, name):
                t = cpool.tile(shape, dtype, tag=name)
                nc.sync.dma_start(out=t[:], in_=src)
                return t

            maskt_s = cload(maskt[:, :], [128, 128], BF, "maskt")
            triut_s = cload(triut[:, :], [128, 128], BF, "triut")
            selb_s = cload(selb[:, :], [NB, NB * 128], BF, "selb")
            ind16_s = cload(ind16[:, :], [128, NB * NB], BF, "ind16")
            ident_s = cload(ident[:, :], [128, 128], F32, "ident")
            bq_c = cload(bqs[:, :], [FPC, 1], F32, "bqc")
            bk_c = cload(bks[:, :], [FPC, 1], F32, "bkc")
            wq_s = cload(wq.rearrange("(kk p) m -> p kk m", p=128),
                         [128, 8, FPC], BF, "wqs")
            wk_s = cload(wk.rearrange("(kk p) m -> p kk m", p=128),
                         [128, 8, FPC], BF, "wks")
            wv_s = cload(wv.rearrange("(kk p) m -> p kk m", p=128),
                         [128, 8, FPC], BF, "wvs")
            wp_s = cload(wp[:, :], [FPC, D], BF, "wps")
            bv_b = cload(bvs[:, :].to_broadcast((128, FPC)), [128, FPC], F32, "bvb")
            bp_b = cload(bp[:, :].to_broadcast((128, D)), [128, D], F32, "bpb")
            gam_b = cload(gam[:, :].to_broadcast((128, D)), [128, D], F32, "gamb")
            bet_b = cload(bet[:, :].to_broadcast((128, D)), [128, D], F32, "betb")
            eps_c = cpool.tile([128, 1], F32, tag="eps")
            nc.vector.memset(eps_c[:], LN_EPS)

            rs_outs = []
            for b in range(B):
                bounce = dram.tile([S, D], F32, tag="bounce")
                rso = nc.dram_tensor(f"rsout{b}", [RPB, D], F32)
                rs_outs.append(rso)
                bs = slice(b * S, (b + 1) * S)

                # ---- K projection -> khT_b [128 feat, 2048 rows] bf16 ----
                xk_b = xbig.tile([128, 8, S], BF, tag="xk")
                nc.sync.dma_start(
                    out=xk_b[:],
                    in_=xkt[:, bs].rearrange("(kk p) n -> p kk n", p=128))
                khT_b = kvp.tile([FPC, S], BF, tag="khT")
                for n in range(4):
                    ps = ppool.tile([128, 512], F32, tag="mm")
                    for kk in range(8):
                        nc.tensor.matmul(
                            ps[:, :], wk_s[:, kk, :],
                            xk_b[:, kk, n * 512:(n + 1) * 512],
                            start=(kk == 0), stop=(kk == 7))
                    nc.vector.tensor_scalar(
                        out=khT_b[:, n * 512:(n + 1) * 512], in0=ps[:, :],
                        scalar1=bk_c[:, :], scalar2=None,
                        op0=mybir.AluOpType.add)

                # ---- V projection -> vh65_b [128 rows, 16 blk, 130] ----
                xv_b = xbig.tile([128, 8, S], BF, tag="xv")
                nc.sync.dma_start(
                    out=xv_b[:],
                    in_=xvt[:, bs].rearrange("(kk p) n -> p kk n", p=128))
                vh65_b = kvp.tile([128, NB, 2 * 65], BF, tag="vh65")
                nc.vector.memset(
                    vh65_b[:].rearrange("p r (h c) -> p r h c", c=65)
                    [:, :, :, 64:65], 1.0)
                for rb in range(NB):
                    psv = ppool.tile([128, 512], F32, tag="mm")
                    for kk in range(8):
                        nc.tensor.matmul(
                            psv[:, 0:FPC],
                            xv_b[:, kk, rb * 128:(rb + 1) * 128],
                            wv_s[:, kk, :],
                            start=(kk == 0), stop=(kk == 7))
                    nc.vector.tensor_add(
                        out=vh65_b[:, rb, :]
                        .rearrange("p (h c) -> p h c", c=65)[:, :, 0:64],
                        in0=psv[:, 0:FPC]
                        .rearrange("p (h c) -> p h c", c=64),
                        in1=bv_b[:, :]
                        .rearrange("p (h c) -> p h c", c=64))

                # block column-sums of vh65 (for masked-region suffix sums)
                psc = ppool.tile([128, 512], F32, tag="mm")
                for rb in range(NB):
                    nc.tensor.matmul(
                        psc[0:NB, 0:130],
                        ind16_s[:, rb * NB:(rb + 1) * NB],
                        vh65_b[:, rb, :],
                        start=(rb == 0), stop=(rb == NB - 1))
                colsum_b = kvp.tile([NB, 2 * 65], BF, tag="colsum")
                nc.vector.tensor_copy(out=colsum_b[:], in_=psc[0:NB, 0:130])

                # ---- Q projection -> qhT_b [128 feat, 2048 rows] (x1/8) ----
                xq_b = xbig.tile([128, 8, S], BF, tag="xq")
                nc.sync.dma_start(
                    out=xq_b[:],
                    in_=xqt[:, bs].rearrange("(kk p) n -> p kk n", p=128))
                qhT_b = kvp.tile([FPC, S], BF, tag="qhT")
                for n in range(4):
                    ps = ppool.tile([128, 512], F32, tag="mm")
                    for kk in range(8):
                        nc.tensor.matmul(
                            ps[:, :], wq_s[:, kk, :],
                            xq_b[:, kk, n * 512:(n + 1) * 512],
                            start=(kk == 0), stop=(kk == 7))
                    nc.vector.tensor_scalar(
                        out=qhT_b[:, n * 512:(n + 1) * 512], in0=ps[:, :],
                        scalar1=1.0 / SCALE, scalar2=bq_c[:, :],
                        op0=mybir.AluOpType.mult, op1=mybir.AluOpType.add)

                # ---- attention, one 128-row query block at a time ----
                for qb in range(NB):
                    ctx_n = ctxp.tile([128, FPC], F32, tag="ctxn")
                    for h01 in range(HPC):
                        hp = slice(h01 * 64, h01 * 64 + 64)
                        vs = slice(h01 * 65, h01 * 65 + 65)
                        ng = qb + 1
                        nt = (ng + 7) // 8
                        # scoresT [kj, qi] for all causal key blocks g<=qb,
                        # packed 8 blocks per PSUM tile, exp'd to SBUF bf16
                        etiles = []
                        for ti in range(nt):
                            w = min(8, ng - ti * 8) * 128
                            st = spool.tile([128, 1024], F32, tag="sc")
                            for gi in range(ti * 8, min(ng, ti * 8 + 8)):
                                col = (gi - ti * 8) * 128
                                nc.tensor.matmul(
                                    st[:, col:col + 128],
                                    khT_b[hp, gi * 128:(gi + 1) * 128],
                                    qhT_b[hp, qb * 128:(qb + 1) * 128],
                                    start=True, stop=True)
                            et = epool.tile([128, 1024], BF, tag="et")
                            nc.scalar.activation(
                                out=et[:, 0:w], in_=st[:, 0:w],
                                func=mybir.ActivationFunctionType.Exp)
                            etiles.append(et)
                        # multiplicative causal mask inside diagonal block
                        dcol = (qb % 8) * 128
                        nc.vector.tensor_mul(
                            out=etiles[-1][:, dcol:dcol + 128],
                            in0=etiles[-1][:, dcol:dcol + 128],
                            in1=maskt_s[:, :])
                        # unnormalized context + denominator [qi, 65]
                        av = apool.tile([128, 512], F32, tag="av")
                        for gi in range(ng):
                            nc.tensor.matmul(
                                av[:, 0:65],
                                etiles[gi // 8][:, (gi % 8) * 128:
                                                (gi % 8) * 128 + 128],
                                vh65_b[:, gi, vs],
                                start=(gi == 0), stop=False)
                        nc.tensor.matmul(
                            av[:, 0:65], triut_s[:, :], vh65_b[:, qb, vs],
                            start=False, stop=False)
                        nc.tensor.matmul(
                            av[:, 0:65],
                            selb_s[:, qb * 128:(qb + 1) * 128],
                            colsum_b[:, vs],
                            start=False, stop=True)
                        rcp = smallp.tile([128, 1], F32, tag="rcp")
                        nc.vector.reciprocal(out=rcp[:], in_=av[:, 64:65])
                        nc.vector.tensor_scalar(
                            out=ctx_n[:, h01 * 64:(h01 + 1) * 64],
                            in0=av[:, 0:64], scalar1=rcp[:, :], scalar2=None,
                            op0=mybir.AluOpType.mult)
                    # transpose ctx -> [feat, qi] for the out-projection
                    trp = apool.tile([128, 512], F32, tag="av")
                    nc.tensor.transpose(trp[:, 0:128], ctx_n[:], ident_s[:])
                    ctxT = ctxp.tile([FPC, 128], BF, tag="ctxT")
                    nc.vector.tensor_copy(out=ctxT[:], in_=trp[:, 0:128])
                    # partial out-projection rows [qi, D]
                    po = fpool.tile([128, D], F32, tag="f4k")
                    for n2 in range(2):
                        pp = ppool.tile([128, 512], F32, tag="mm")
                        nc.tensor.matmul(
                            pp[:, :], ctxT[:],
                            wp_s[:, n2 * 512:(n2 + 1) * 512],
                            start=True, stop=True)
                        nc.vector.tensor_copy(
                            out=po[:, n2 * 512:(n2 + 1) * 512], in_=pp[:, :])
                    nc.sync.dma_start(
                        out=bounce[qb * 128:(qb + 1) * 128, :], in_=po[:])

                # ---- sum partials across cores; rank r gets its rows ----
                nc.gpsimd.collective_compute(
                    "ReduceScatter", mybir.AluOpType.add,
                    replica_groups=[list(range(NCORES))],
                    ins=[bounce[:].opt()], outs=[rso[:].opt()])

            # ---- residual + LayerNorm on this core's row shard ----
            for b in range(B):
                for t in range(RPB // 128):
                    ld = fpool.tile([128, D], F32, tag="f4k")
                    nc.sync.dma_start(out=ld[:],
                                      in_=rs_outs[b][t * 128:(t + 1) * 128, :])
                    rs_t = fpool.tile([128, D], F32, tag="f4k")
                    nc.sync.dma_start(
                        out=rs_t[:],
                        in_=res[b * RPB + t * 128:b * RPB + (t + 1) * 128, :])
                    nc.vector.tensor_add(out=ld[:], in0=ld[:], in1=bp_b[:])
                    nc.vector.tensor_add(out=ld[:], in0=ld[:], in1=rs_t[:])
                    stats = smallp.tile([128, 2, 6], F32, tag="stats")
                    for c2 in range(2):
                        nc.vector.bn_stats(out=stats[:, c2, :],
                                           in_=ld[:, c2 * 512:(c2 + 1) * 512])
                    mv = smallp.tile([128, 2], F32, tag="mv")
                    nc.vector.bn_aggr(out=mv[:], in_=stats[:])
                    lnv = smallp.tile([128, 1], F32, tag="lnv")
                    nc.scalar.activation(
                        out=lnv[:], in_=mv[:, 1:2],
                        func=mybir.ActivationFunctionType.Ln, bias=eps_c[:, :])
                    rstd = smallp.tile([128, 1], F32, tag="rstd")
                    nc.scalar.activation(
                        out=rstd[:], in_=lnv[:],
                        func=mybir.ActivationFunctionType.Exp, scale=-0.5)
                    yt = fpool.tile([128, D], F32, tag="f4k")
                    nc.vector.tensor_scalar(
                        out=yt[:], in0=ld[:], scalar1=mv[:, 0:1],
                        scalar2=rstd[:, :], op0=mybir.AluOpType.subtract,
                        op1=mybir.AluOpType.mult)
                    nc.vector.tensor_mul(out=yt[:], in0=yt[:], in1=gam_b[:])
                    nc.vector.tensor_add(out=yt[:], in0=yt[:], in1=bet_b[:])
                    nc.sync.dma_start(
                        out=y[b * RPB + t * 128:b * RPB + (t + 1) * 128, :],
                        in_=yt[:])

    nc.compile()
    return nc


def _host_inputs(q, k, v, Wq, bq, Wk, bk, Wv, bv, Wp, bp, gamma, beta):
    """Build the 8 per-core input maps from the full-size inputs."""
    qf = np.asarray(q, np.float32).reshape(B * S, D)
    kf = np.asarray(k, np.float32).reshape(B * S, D)
    vf = np.asarray(v, np.float32).reshape(B * S, D)
    xqt = np.ascontiguousarray(qf.T).astype(BF16)
    xkt = np.ascontiguousarray(kf.T).astype(BF16)
    xvt = np.ascontiguousarray(vf.T).astype(BF16)

    Wq = np.asarray(Wq, np.float32)
    Wk = np.asarray(Wk, np.float32)
    Wv = np.asarray(Wv, np.float32)
    Wp = np.asarray(Wp, np.float32)
    bq = np.asarray(bq, np.float32)
    bk = np.asarray(bk, np.float32)
    bv = np.asarray(bv, np.float32)
    bp = np.asarray(bp, np.float32)
    gamma = np.asarray(gamma, np.float32)
    beta = np.asarray(beta, np.float32)

    ii, jj = np.meshgrid(np.arange(128), np.arange(128), indexing="ij")
    maskt = (ii <= jj).astype(BF16)          # [kj, qi]: keep j <= i
    triut = (ii > jj).astype(BF16)           # [kj, qi]: strict upper ones
    selb = np.zeros((NB, NB * 128), BF16)
    for qb in range(NB):
        selb[qb + 1:, qb * 128:(qb + 1) * 128] = 1
    ind16 = np.zeros((128, NB * NB), BF16)
    for rb in range(NB):
        ind16[:, rb * NB + rb] = 1
    ident = np.eye(128, dtype=np.float32)

    in_maps = []
    for r in range(NCORES):
        cs = slice(r * FPC, (r + 1) * FPC)
        rows = np.concatenate(
            [np.arange(b * S + r * RPB, b * S + (r + 1) * RPB)
             for b in range(B)])
        in_maps.append({
            "xqt": xqt, "xkt": xkt, "xvt": xvt,
            "wq": Wq[:, cs].astype(BF16),
            "wk": Wk[:, cs].astype(BF16),
            "wv": Wv[:, cs].astype(BF16),
            "wp": np.ascontiguousarray(Wp[cs, :]).astype(BF16),
            "bqs": (bq[cs] / SCALE).reshape(FPC, 1).astype(np.float32),
            "bks": bk[cs].reshape(FPC, 1).astype(np.float32),
            "bvs": bv[cs].reshape(1, FPC).astype(np.float32),
            "bp": bp.reshape(1, D),
            "gam": gamma.reshape(1, D),
            "bet": beta.reshape(1, D),
            "res": np.ascontiguousarray(qf[rows]),
            "maskt": maskt, "triut": triut, "selb": selb,
            "ind16": ind16, "ident": ident,
        })
    return in_maps


def _assemble(results):
    out = np.empty((B * S, D), np.float32)
    for r in range(NCORES):
        yr = results[r]["y"]
        for b in range(B):
            out[b * S + r * RPB:b * S + (r + 1) * RPB] = \
                yr[b * RPB:(b + 1) * RPB]
    return out.reshape(B, S, D)


def kernel(**inputs) -> np.ndarray:
    if "nc" not in _CACHE:
        _CACHE["nc"] = _build_nc()
    nc = _CACHE["nc"]
    in_maps = _host_inputs(**inputs)
    res = run_bass_kernel_spmd(nc, in_maps, core_ids=list(range(NCORES)))
    return _assemble(res.results)


def kernel_profiled(**inputs):
    """Like kernel(), but captures an NTFF profile. Returns (out, result)."""
    if "nc" not in _CACHE:
        _CACHE["nc"] = _build_nc()
    nc = _CACHE["nc"]
    in_maps = _host_inputs(**inputs)
    res = run_bass_kernel_spmd(nc, in_maps, core_ids=list(range(NCORES)),
                               trace=True)
    return _assemble(res.results), res


if __name__ == "__main__":
    rng = np.random.default_rng(0)
    std = 1.0 / np.sqrt(D)
    inp = {
        "q": rng.standard_normal((B, S, D), np.float32),
        "k": rng.standard_normal((B, S, D), np.float32),
        "v": rng.standard_normal((B, S, D), np.float32),
        "Wq": rng.standard_normal((D, D), np.float32) * std,
        "bq": np.zeros(D, np.float32),
        "Wk": rng.standard_normal((D, D), np.float32) * std,
        "bk": np.zeros(D, np.float32),
        "Wv": rng.standard_normal((D, D), np.float32) * std,
        "bv": np.zeros(D, np.float32),
        "Wp": rng.standard_normal((D, D), np.float32) * std,
        "bp": np.zeros(D, np.float32),
        "gamma": np.ones(D, np.float32),
        "beta": np.zeros(D, np.float32),
    }
    out = kernel(**inp)
    print("kernel output:", out.shape, out.dtype)
